# revision 12
# baseline (speedup 1.0000x reference)
"""MetapathAttentionLayer Trainium2 kernel (v3).

Math (per node n):
    scores[n, m] = sum_d x[m, n, d] * W[d, m]
    att = softmax(relu(scores), axis=m)      (8 metapaths)
    out[n, :] = elu(sum_m att[n, m] * x[m, n, :])

Strategy: shard nodes across 8 cores (data parallel), 12544 padded nodes
per core. SBUF layout: partition p = (m, r), m = metapath (8), r =
node-residue (16); node n = r*784 + c for chunk-column c in [0, 784).

Per group of ~56 chunk-columns:
  - scores: DVE bf16 multiply (2x mode) then a d-reduction tree. The tree
    is split by chunk range: the first kd chunks run L1-L3 on DVE, the
    rest on GPSIMD(Pool) — balancing the two engines (both ~85us/core).
    L4-L7 run on DVE. relu+exp on ACT give e1 = exp(relu(s)) directly.
  - softmax denominator: one PE matmul with a residue-replication matrix
    sums e1 over m and replicates Z to all partitions; 1/Z via DVE fast
    reciprocal; att = e1 * invZ on DVE.
  - A-matrix (att * delta(r==i), layout [p, (i, c)]) via 16 ACT Copy ops
    with a per-partition 0/1 scale vector (ACT has slack).
  - pooling: one PE matmul per chunk (lhsT = X-chunk [128, 128d],
    rhs = strided A-slice [128, 16]) -> U[d, 16] in PSUM.
  - elu via PSUM accumulation: elu(u) = u + relu(-u) + exp(-relu(-u)) - 1.
    ACT computes b = relu(-U), c2 = exp(-b); two identity matmuls on PE
    accumulate b and c2 into the U bank; final ACT Copy applies bias -1
    and writes bf16.
  - store d-major [128=d, (c, i)] straight to DRAM (4KB-class descriptors,
    no transpose); host un-permutes to [n, d] for free.
"""

import numpy as np
import ml_dtypes

import concourse.bass as bass
import concourse.tile as tile
from concourse import bacc, mybir, library_config
import concourse.bass_utils as bass_utils

F32 = mybir.dt.float32
BF16 = mybir.dt.bfloat16
I16 = mybir.dt.int16
ALU = mybir.AluOpType
ACTF = mybir.ActivationFunctionType

NMETA = 8
N = 100000
D = 128
NCORES = 8
NC_RAW = N // NCORES          # 12500 nodes per core
R = 16                        # node-residue groups on partitions
NB = 784                      # chunk-columns per residue: 16*784 = 12544
NC_PAD = R * NB               # padded nodes per core


def kernel_body(tc, out_d, x_d, wb_d, maskf_d, rep_d, eye_d, sidx_d,
                split=0.47, e47="dve", a_eng="alt", out_q="act", out_delay=3,
                bufs_x=4, bufs_p=2, bufs_tree=2, bufs_small=10,
                bufs_a=2, bufs_bc=2, bufs_y=4, sizes=None):
    nc = tc.nc
    if sizes is None:
        sizes = [56] * 14
    assert sum(sizes) == NB
    offs = [0]
    for s_ in sizes:
        offs.append(offs[-1] + s_)
    NGv = len(sizes)
    from contextlib import ExitStack
    with ExitStack() as ctx:
        const = ctx.enter_context(tc.tile_pool(name="const", bufs=1))
        xpool = ctx.enter_context(tc.tile_pool(name="x", bufs=bufs_x))
        ppool = ctx.enter_context(tc.tile_pool(name="p", bufs=bufs_p))
        tpool = ctx.enter_context(tc.tile_pool(name="tree", bufs=bufs_tree))
        spool = ctx.enter_context(tc.tile_pool(name="small", bufs=bufs_small))
        apool = ctx.enter_context(tc.tile_pool(name="amat", bufs=bufs_a))
        bcpool = ctx.enter_context(tc.tile_pool(name="bc", bufs=bufs_bc))
        ypool = ctx.enter_context(tc.tile_pool(name="y", bufs=bufs_y))
        psU = ctx.enter_context(tc.tile_pool(name="psU", bufs=2, space="PSUM"))
        psZ = ctx.enter_context(tc.tile_pool(name="psZ", bufs=2, space="PSUM"))

        cst = {}

        def load_consts():
            wb_t = const.tile([128, D], BF16, tag="wb")
            nc.sync.dma_start(wb_t[:], wb_d[:])
            maskf_t = const.tile([128, R], F32, tag="maskf")
            nc.sync.dma_start(maskf_t[:], maskf_d[:])
            rep_t = const.tile([128, 128], BF16, tag="rep")
            nc.sync.dma_start(rep_t[:], rep_d[:])
            eye_t = const.tile([128, 128], BF16, tag="eye")
            nc.sync.dma_start(eye_t[:], eye_d[:])
            sidx_t = const.tile([128, max(sizes)], I16, tag="sidx")
            nc.sync.dma_start(sidx_t[:], sidx_d[:])
            cst.update(wb=wb_t, maskf=maskf_t, rep=rep_t, eye=eye_t,
                       sidx=sidx_t)
            if a_eng != "dve":
                nc.gpsimd.load_library(library_config.local_scatter)

        dma_eng = {"act": nc.scalar, "sp": nc.sync, "pool": nc.gpsimd,
                   "dve": nc.vector}[out_q]
        st = [dict() for _ in range(NGv)]
        pending = []

        def load(g):
            ct = sizes[g]
            X = xpool.tile([128, ct * D], BF16, tag="X")
            nc.sync.dma_start(X[:], x_d[:, offs[g] * D:(offs[g] + ct) * D])
            st[g]["Xv"] = X[:].rearrange("p (c d) -> p c d", c=ct)

        def mult(g):
            ct = sizes[g]
            Xv = st[g]["Xv"]
            P = ppool.tile([128, ct * D], BF16, tag="P")
            Pv = P[:].rearrange("p (c d) -> p c d", c=ct)
            nc.vector.tensor_tensor(
                out=Pv, in0=Xv,
                in1=cst["wb"][:].unsqueeze(1).broadcast_to([128, ct, D]),
                op=ALU.mult)
            st[g]["Pv"] = Pv

        def trees(g):
            # L1-L3: chunks [0:kd] on DVE, [kd:ct] on Pool; both write P3.
            ct = sizes[g]
            kd = ct - int(round(ct * split))
            Pv = st[g].pop("Pv")
            P1 = tpool.tile([128, ct * 64], BF16, tag="P1")
            P1v = P1[:].rearrange("p (c d) -> p c d", c=ct)
            P2 = tpool.tile([128, ct * 32], BF16, tag="P2")
            P2v = P2[:].rearrange("p (c d) -> p c d", c=ct)
            P3 = tpool.tile([128, ct * 16], BF16, tag="P3")
            P3v = P3[:].rearrange("p (c d) -> p c d", c=ct)
            if kd > 0:
                nc.vector.tensor_tensor(
                    out=P1v[:, 0:kd, :], in0=Pv[:, 0:kd, 0:64],
                    in1=Pv[:, 0:kd, 64:128], op=ALU.add)
                nc.vector.tensor_tensor(
                    out=P2v[:, 0:kd, :], in0=P1v[:, 0:kd, 0:32],
                    in1=P1v[:, 0:kd, 32:64], op=ALU.add)
                nc.vector.tensor_tensor(
                    out=P3v[:, 0:kd, :], in0=P2v[:, 0:kd, 0:16],
                    in1=P2v[:, 0:kd, 16:32], op=ALU.add)
            if kd < ct:
                nc.gpsimd.tensor_tensor(
                    out=P1v[:, kd:ct, :], in0=Pv[:, kd:ct, 0:64],
                    in1=Pv[:, kd:ct, 64:128], op=ALU.add)
                nc.gpsimd.tensor_tensor(
                    out=P2v[:, kd:ct, :], in0=P1v[:, kd:ct, 0:32],
                    in1=P1v[:, kd:ct, 32:64], op=ALU.add)
                nc.gpsimd.tensor_tensor(
                    out=P3v[:, kd:ct, :], in0=P2v[:, kd:ct, 0:16],
                    in1=P2v[:, kd:ct, 16:32], op=ALU.add)
            st[g]["P3v"] = P3v

        def l47(g):
            ct = sizes[g]
            eng = nc.vector if e47 == "dve" else nc.gpsimd
            P3v = st[g].pop("P3v")
            P4 = tpool.tile([128, ct * 8], BF16, tag="P4")
            P4v = P4[:].rearrange("p (c d) -> p c d", c=ct)
            eng.tensor_tensor(out=P4v, in0=P3v[:, :, 0:8], in1=P3v[:, :, 8:16],
                              op=ALU.add)
            P5 = tpool.tile([128, ct * 4], BF16, tag="P5")
            P5v = P5[:].rearrange("p (c d) -> p c d", c=ct)
            eng.tensor_tensor(out=P5v, in0=P4v[:, :, 0:4], in1=P4v[:, :, 4:8],
                              op=ALU.add)
            P6 = tpool.tile([128, ct * 2], BF16, tag="P6")
            P6v = P6[:].rearrange("p (c d) -> p c d", c=ct)
            eng.tensor_tensor(out=P6v, in0=P5v[:, :, 0:2], in1=P5v[:, :, 2:4],
                              op=ALU.add)
            scores = spool.tile([128, ct], F32, tag="scores")
            eng.tensor_tensor(out=scores[:].unsqueeze(2), in0=P6v[:, :, 0:1],
                              in1=P6v[:, :, 1:2], op=ALU.add)
            st[g]["scores"] = scores

        def softmax_head(g):
            ct = sizes[g]
            rs = spool.tile([128, ct], F32, tag="rs")
            nc.scalar.activation(rs[:], st[g].pop("scores")[:], ACTF.Relu)
            e1 = spool.tile([128, ct], BF16, tag="e1")
            nc.scalar.activation(e1[:], rs[:], ACTF.Exp)
            Z = psZ.tile([128, ct], F32, tag="Z")
            nc.tensor.matmul(out=Z[:], lhsT=cst["rep"][:], rhs=e1[:],
                             start=True, stop=True)
            st[g]["e1"] = e1
            st[g]["Z"] = Z

        def att_abuild(g):
            ct = sizes[g]
            inv = spool.tile([128, ct], F32, tag="inv")
            nc.vector.reciprocal_approx_fast(out=inv[:], in_=st[g].pop("Z")[:])
            att = spool.tile([128, ct], BF16, tag="att")
            nc.vector.tensor_tensor(out=att[:], in0=st[g].pop("e1")[:],
                                    in1=inv[:], op=ALU.mult)
            A = apool.tile([128, R * ct], BF16, tag="A")
            eng = a_eng if a_eng != "alt" else ("dve" if g % 2 == 0 else "pool")
            if eng == "dve":
                # A in (i, c) layout: A[p, i*ct + c] = att[p, c]*maskf[p, i]
                for i in range(R):
                    nc.vector.tensor_scalar(
                        A[:, i * ct:(i + 1) * ct], att[:],
                        cst["maskf"][:, i:i + 1], None, ALU.mult)
                st[g]["Av"] = A[:].rearrange("p (i c) -> p c i", i=R)
            else:
                # A in (c, i) layout via gpsimd scatter: A[p, c*R + r(p)]
                nc.gpsimd.local_scatter(A[:], att[:], cst["sidx"][:, 0:ct],
                                        channels=128, num_elems=ct * R,
                                        num_idxs=ct)
                st[g]["Av"] = A[:].rearrange("p (c i) -> p c i", i=R)

        def pool_mm(g):
            ct = sizes[g]
            Xv = st[g].pop("Xv")
            Av = st[g].pop("Av")
            U = psU.tile([128, ct * R], F32, tag="U")
            # PSUM accumulation-group discipline: start=True zeroes (lazily)
            # the whole 2KB zero region, so only the first matmul touching
            # each 512-f32 bank may set it; the ELU accumulates below then
            # add into live banks, and the last one per bank stops the group.
            for c in range(ct):
                nc.tensor.matmul(
                    out=U[:, c * R:(c + 1) * R],
                    lhsT=Xv[:, c, :],
                    rhs=Av[:, c, :],
                    start=(c * R) % 512 == 0, stop=False)
            st[g]["U"] = U

        def elu(g):
            ct = sizes[g]
            U = st[g]["U"]
            b = bcpool.tile([128, ct * R], BF16, tag="b")
            nc.scalar.activation(b[:], U[:], ACTF.Relu, scale=-1.0)
            c2 = bcpool.tile([128, ct * R], BF16, tag="c2")
            nc.scalar.activation(c2[:], b[:], ACTF.Exp, scale=-1.0)
            # accumulate b and c2 into the U PSUM banks via identity matmuls,
            # in 512-column (one f32 PSUM bank) aligned pieces; the final
            # accumulate per bank closes the accumulation group.
            for j in range(0, ct * R, 512):
                e_ = min(ct * R, j + 512)
                nc.tensor.matmul(out=U[:, j:e_], lhsT=cst["eye"][:],
                                 rhs=c2[:, j:e_], start=False, stop=False)
                nc.tensor.matmul(out=U[:, j:e_], lhsT=cst["eye"][:],
                                 rhs=b[:, j:e_], start=False, stop=True)

        def final(g):
            ct = sizes[g]
            U = st[g].pop("U")
            y = ypool.tile([128, ct * R], BF16, tag="y")
            nc.scalar.activation(y[:], U[:], ACTF.Copy, bias=-1.0)
            pending.append((out_d[:, offs[g] * R:(offs[g] + ct) * R], y[:]))

        def ok(g):
            return 0 <= g < NGv

        load(0)
        load_consts()
        for it in range(NGv + 2):
            if ok(it + 1):
                load(it + 1)
            if pending and it >= out_delay:
                dst, src_ = pending.pop(0)
                dma_eng.dma_start(dst, src_)
            if ok(it):
                mult(it)
                trees(it)
                l47(it)
                softmax_head(it)
            if ok(it - 1):
                att_abuild(it - 1)
                pool_mm(it - 1)
                elu(it - 1)
            if ok(it - 2):
                final(it - 2)
        for dst, src_ in pending:
            dma_eng.dma_start(dst, src_)


def host_inputs(x_np, w_np):
    """Build per-core input maps from full fp32 inputs."""
    in_maps = []
    w_bf = w_np.astype(ml_dtypes.bfloat16)          # [D, NMETA]
    # wb[(m,r), d] = W[d, m]
    wb = np.ascontiguousarray(np.repeat(w_bf.T, R, axis=0))     # [128, D]
    maskf = np.zeros((128, R), dtype=np.float32)
    for p in range(128):
        maskf[p, p % R] = 1.0
    rep = np.zeros((128, 128), dtype=ml_dtypes.bfloat16)
    for p in range(128):
        for m2 in range(NMETA):
            rep[p, m2 * R + (p % R)] = 1.0
    eye = np.eye(128, dtype=ml_dtypes.bfloat16)
    sidx = np.zeros((128, 56), dtype=np.int16)
    for p in range(128):
        for c in range(56):
            sidx[p, c] = c * R + (p % R)

    nc_raw = x_np.shape[1] // NCORES
    for core in range(NCORES):
        xs = x_np[:, core * nc_raw:(core + 1) * nc_raw, :]
        xp = np.zeros((NMETA, NC_PAD, D), dtype=ml_dtypes.bfloat16)
        xp[:, :nc_raw, :] = xs.astype(ml_dtypes.bfloat16)
        # xb[(m, r), (c, d)] = x[m, r*NB + c, d]
        xb = np.ascontiguousarray(
            xp.reshape(NMETA, R, NB * D).reshape(128, NB * D))
        in_maps.append({"x": xb, "wb": wb, "maskf": maskf, "rep": rep,
                        "eye": eye, "sidx": sidx})
    return in_maps


_CACHE = {}


def build(**kw):
    key = tuple(sorted((k, tuple(v) if isinstance(v, list) else v)
                       for k, v in kw.items()))
    if key in _CACHE:
        return _CACHE[key]
    nc = bacc.Bacc("TRN2", target_bir_lowering=False, debug=False,
                   num_devices=NCORES)
    x = nc.dram_tensor("x", [128, NB * D], BF16, kind="ExternalInput").ap()
    wb = nc.dram_tensor("wb", [128, D], BF16, kind="ExternalInput").ap()
    maskf = nc.dram_tensor("maskf", [128, R], F32, kind="ExternalInput").ap()
    rep = nc.dram_tensor("rep", [128, 128], BF16, kind="ExternalInput").ap()
    eye = nc.dram_tensor("eye", [128, 128], BF16, kind="ExternalInput").ap()
    sidx = nc.dram_tensor("sidx", [128, 56], I16, kind="ExternalInput").ap()
    # out is d-major: out[d, c*R + i] = y[node(r=i, c), d]
    out = nc.dram_tensor("out", [128, NB * R], BF16,
                         kind="ExternalOutput").ap()
    with tile.TileContext(nc) as tc:
        kernel_body(tc, out, x, wb, maskf, rep, eye, sidx, **kw)
    nc.compile()
    _CACHE[key] = nc
    return nc


def unpermute(o_core):
    # o_core [128=d, NB*R] with col j = c*R + i  ->  [NC_PAD, D], n = i*NB + c
    return np.ascontiguousarray(
        np.asarray(o_core).reshape(D, NB, R).transpose(2, 1, 0)
        .reshape(NC_PAD, D))


def run(input, W, trace=False, _build_kw=None, **trace_kwargs):
    x_np = np.asarray(input, dtype=np.float32)
    w_np = np.asarray(W, dtype=np.float32)
    nc = build(**(_build_kw or {}))
    in_maps = host_inputs(x_np, w_np)
    res = bass_utils.run_bass_kernel_spmd(
        nc, in_maps, core_ids=list(range(NCORES)), trace=trace, **trace_kwargs)
    nc_raw = x_np.shape[1] // NCORES
    full = np.concatenate(
        [unpermute(res.results[c]["out"])[:nc_raw] for c in range(NCORES)],
        axis=0).astype(np.float32)
    return full, res


def kernel(input, W):
    out, _ = run(input, W, trace=False)
    return out


# revision 19
# speedup vs baseline: 1.4159x; 1.4159x over previous
"""MetapathAttentionLayer Trainium2 kernel (v3).

Math (per node n):
    scores[n, m] = sum_d x[m, n, d] * W[d, m]
    att = softmax(relu(scores), axis=m)      (8 metapaths)
    out[n, :] = elu(sum_m att[n, m] * x[m, n, :])

Strategy: shard nodes across 8 cores (data parallel), 12544 padded nodes
per core. SBUF layout: partition p = (m, r), m = metapath (8), r =
node-residue (16); node n = r*784 + c for chunk-column c in [0, 784).

Per group of ~56 chunk-columns:
  - scores: DVE bf16 multiply (2x mode) then a d-reduction tree. The tree
    is split by chunk range: the first kd chunks run L1-L3 on DVE, the
    rest on GPSIMD(Pool) — balancing the two engines (both ~85us/core).
    L4-L7 run on DVE. relu+exp on ACT give e1 = exp(relu(s)) directly.
  - softmax denominator: one PE matmul with a residue-replication matrix
    sums e1 over m and replicates Z to all partitions; 1/Z via DVE fast
    reciprocal; att = e1 * invZ on DVE.
  - A-matrix (att * delta(r==i), layout [p, (i, c)]) via 16 ACT Copy ops
    with a per-partition 0/1 scale vector (ACT has slack).
  - pooling: one PE matmul per chunk (lhsT = X-chunk [128, 128d],
    rhs = strided A-slice [128, 16]) -> U[d, 16] in PSUM.
  - elu via PSUM accumulation: elu(u) = u + relu(-u) + exp(-relu(-u)) - 1.
    ACT computes b = relu(-U), c2 = exp(-b); two identity matmuls on PE
    accumulate b and c2 into the U bank; final ACT Copy applies bias -1
    and writes bf16.
  - store d-major [128=d, (c, i)] straight to DRAM (4KB-class descriptors,
    no transpose); host un-permutes to [n, d] for free.
"""

import numpy as np
import ml_dtypes

import concourse.bass as bass
import concourse.tile as tile
from concourse import bacc, mybir, library_config
import concourse.bass_utils as bass_utils

F32 = mybir.dt.float32
BF16 = mybir.dt.bfloat16
I16 = mybir.dt.int16
ALU = mybir.AluOpType
ACTF = mybir.ActivationFunctionType

NMETA = 8
N = 100000
D = 128
NCORES = 8
NC_RAW = N // NCORES          # 12500 nodes per core
R = 16                        # node-residue groups on partitions
NB = 784                      # chunk-columns per residue: 16*784 = 12544
NC_PAD = R * NB               # padded nodes per core


def kernel_body(tc, out_d, x_d, wb_d, maskf_d, rep_d, eye_d, sidx_d,
                pe_passes=128, a_eng="pool", out_q="act",
                out_delay=3, bufs_x=4, bufs_p=2, bufs_tree=2, bufs_small=10,
                bufs_a=2, bufs_bc=2, bufs_y=4, sizes=None):
    nc = tc.nc
    if sizes is None:
        sizes = [56] * 14
    assert sum(sizes) == NB
    offs = [0]
    for s_ in sizes:
        offs.append(offs[-1] + s_)
    NGv = len(sizes)
    from contextlib import ExitStack
    with ExitStack() as ctx:
        const = ctx.enter_context(tc.tile_pool(name="const", bufs=1))
        xpool = ctx.enter_context(tc.tile_pool(name="x", bufs=bufs_x))
        ppool = ctx.enter_context(tc.tile_pool(name="p", bufs=bufs_p))
        tpool = ctx.enter_context(tc.tile_pool(name="tree", bufs=bufs_tree))
        spool = ctx.enter_context(tc.tile_pool(name="small", bufs=bufs_small))
        apool = ctx.enter_context(tc.tile_pool(name="amat", bufs=bufs_a))
        bcpool = ctx.enter_context(tc.tile_pool(name="bc", bufs=bufs_bc))
        ypool = ctx.enter_context(tc.tile_pool(name="y", bufs=bufs_y))
        psU = ctx.enter_context(tc.tile_pool(name="psU", bufs=2, space="PSUM"))
        psZ = ctx.enter_context(tc.tile_pool(name="psZ", bufs=2, space="PSUM"))
        psS = ctx.enter_context(tc.tile_pool(name="psS", bufs=2, space="PSUM"))

        cst = {}

        def load_consts():
            wb_t = const.tile([128, D], BF16, tag="wb")
            nc.sync.dma_start(wb_t[:], wb_d[:])
            maskf_t = const.tile([128, R], F32, tag="maskf")
            nc.sync.dma_start(maskf_t[:], maskf_d[:])
            rep_t = const.tile([128, 128], BF16, tag="rep")
            nc.sync.dma_start(rep_t[:], rep_d[:])
            eye_t = const.tile([128, 128], BF16, tag="eye")
            nc.sync.dma_start(eye_t[:], eye_d[:])
            sidx_t = const.tile([128, max(sizes)], I16, tag="sidx")
            nc.sync.dma_start(sidx_t[:], sidx_d[:])
            cst.update(wb=wb_t, maskf=maskf_t, rep=rep_t, eye=eye_t,
                       sidx=sidx_t)
            if a_eng != "dve":
                nc.gpsimd.load_library(library_config.local_scatter)

        dma_eng = {"act": nc.scalar, "sp": nc.sync, "pool": nc.gpsimd,
                   "dve": nc.vector}[out_q]
        st = [dict() for _ in range(NGv)]
        pending = []

        def load(g):
            ct = sizes[g]
            X = xpool.tile([128, ct * D], BF16, tag="X")
            nc.sync.dma_start(X[:], x_d[:, offs[g] * D:(offs[g] + ct) * D])
            st[g]["Xv"] = X[:].rearrange("p (c d) -> p c d", c=ct)

        def mult(g):
            ct = sizes[g]
            Xv = st[g]["Xv"]
            P = ppool.tile([128, ct * D], BF16, tag="P")
            Pv = P[:].rearrange("p (c d) -> p c d", c=ct)
            nc.vector.tensor_tensor(
                out=Pv, in0=Xv,
                in1=cst["wb"][:].unsqueeze(1).broadcast_to([128, ct, D]),
                op=ALU.mult)
            st[g]["Pv"] = Pv

        def scores_stage(g):
            # d-reduction entirely on PE: optional DVE pre-levels halve the
            # slice count, then PE accumulates the remaining single-column
            # slices into a [128, ct] f32 PSUM tile via identity matmuls.
            ct = sizes[g]
            Sv = st[g].pop("Pv")
            w = 128
            if pe_passes <= 64:
                P1 = tpool.tile([128, ct * 64], BF16, tag="P1")
                P1v = P1[:].rearrange("p (c d) -> p c d", c=ct)
                nc.vector.tensor_tensor(out=P1v, in0=Sv[:, :, 0:64],
                                        in1=Sv[:, :, 64:128], op=ALU.add)
                Sv, w = P1v, 64
            if pe_passes <= 32:
                P2 = tpool.tile([128, ct * 32], BF16, tag="P2")
                P2v = P2[:].rearrange("p (c d) -> p c d", c=ct)
                nc.vector.tensor_tensor(out=P2v, in0=Sv[:, :, 0:32],
                                        in1=Sv[:, :, 32:64], op=ALU.add)
                Sv, w = P2v, 32
            S = psS.tile([128, ct], F32, tag="S")
            for k in range(w):
                nc.tensor.matmul(out=S[:], lhsT=cst["eye"][:],
                                 rhs=Sv[:, :, k:k + 1],
                                 start=(k == 0), stop=(k == w - 1))
            st[g]["scores"] = S

        def softmax_head(g):
            ct = sizes[g]
            rs = spool.tile([128, ct], F32, tag="rs")
            nc.scalar.activation(rs[:], st[g].pop("scores")[:], ACTF.Relu)
            e1 = spool.tile([128, ct], BF16, tag="e1")
            nc.scalar.activation(e1[:], rs[:], ACTF.Exp)
            Z = psZ.tile([128, ct], F32, tag="Z")
            nc.tensor.matmul(out=Z[:], lhsT=cst["rep"][:], rhs=e1[:],
                             start=True, stop=True)
            st[g]["e1"] = e1
            st[g]["Z"] = Z

        def att_abuild(g):
            ct = sizes[g]
            inv = spool.tile([128, ct], F32, tag="inv")
            nc.vector.reciprocal_approx_fast(out=inv[:], in_=st[g].pop("Z")[:])
            att = spool.tile([128, ct], BF16, tag="att")
            nc.vector.tensor_tensor(out=att[:], in0=st[g].pop("e1")[:],
                                    in1=inv[:], op=ALU.mult)
            A = apool.tile([128, R * ct], BF16, tag="A")
            eng = a_eng if a_eng != "alt" else ("dve" if g % 2 == 0 else "pool")
            if eng == "dve":
                # A in (i, c) layout: A[p, i*ct + c] = att[p, c]*maskf[p, i]
                for i in range(R):
                    nc.vector.tensor_scalar(
                        A[:, i * ct:(i + 1) * ct], att[:],
                        cst["maskf"][:, i:i + 1], None, ALU.mult)
                st[g]["Av"] = A[:].rearrange("p (i c) -> p c i", i=R)
            else:
                # A in (c, i) layout via gpsimd scatter: A[p, c*R + r(p)]
                nc.gpsimd.local_scatter(A[:], att[:], cst["sidx"][:, 0:ct],
                                        channels=128, num_elems=ct * R,
                                        num_idxs=ct)
                st[g]["Av"] = A[:].rearrange("p (c i) -> p c i", i=R)

        def pool_mm(g):
            ct = sizes[g]
            Xv = st[g].pop("Xv")
            Av = st[g].pop("Av")
            U = psU.tile([128, ct * R], F32, tag="U")
            # PSUM accumulation-group discipline: start=True zeroes (lazily)
            # the whole 2KB zero region, so only the first matmul touching
            # each 512-f32 bank may set it; the ELU accumulates below then
            # add into live banks, and the last one per bank stops the group.
            for c in range(ct):
                nc.tensor.matmul(
                    out=U[:, c * R:(c + 1) * R],
                    lhsT=Xv[:, c, :],
                    rhs=Av[:, c, :],
                    start=(c * R) % 512 == 0, stop=False)
            st[g]["U"] = U

        def elu(g):
            ct = sizes[g]
            U = st[g]["U"]
            b = bcpool.tile([128, ct * R], BF16, tag="b")
            nc.scalar.activation(b[:], U[:], ACTF.Relu, scale=-1.0)
            c2 = bcpool.tile([128, ct * R], BF16, tag="c2")
            nc.scalar.activation(c2[:], b[:], ACTF.Exp, scale=-1.0)
            # accumulate b and c2 into the U PSUM banks via identity matmuls,
            # in 512-column (one f32 PSUM bank) aligned pieces; the final
            # accumulate per bank closes the accumulation group.
            for j in range(0, ct * R, 512):
                e_ = min(ct * R, j + 512)
                nc.tensor.matmul(out=U[:, j:e_], lhsT=cst["eye"][:],
                                 rhs=c2[:, j:e_], start=False, stop=False)
                nc.tensor.matmul(out=U[:, j:e_], lhsT=cst["eye"][:],
                                 rhs=b[:, j:e_], start=False, stop=True)

        def final(g):
            ct = sizes[g]
            U = st[g].pop("U")
            y = ypool.tile([128, ct * R], BF16, tag="y")
            nc.scalar.activation(y[:], U[:], ACTF.Copy, bias=-1.0)
            pending.append((out_d[:, offs[g] * R:(offs[g] + ct) * R], y[:]))

        def ok(g):
            return 0 <= g < NGv

        load(0)
        load_consts()
        for it in range(NGv + 2):
            if ok(it + 1):
                load(it + 1)
            if pending and it >= out_delay:
                dst, src_ = pending.pop(0)
                dma_eng.dma_start(dst, src_)
            if ok(it - 1):
                att_abuild(it - 1)
                pool_mm(it - 1)
                elu(it - 1)
            if ok(it - 2):
                final(it - 2)
            if ok(it):
                mult(it)
                scores_stage(it)
                softmax_head(it)
        for dst, src_ in pending:
            dma_eng.dma_start(dst, src_)


def host_inputs(x_np, w_np):
    """Build per-core input maps from full fp32 inputs."""
    in_maps = []
    w_bf = w_np.astype(ml_dtypes.bfloat16)          # [D, NMETA]
    # wb[(m,r), d] = W[d, m]
    wb = np.ascontiguousarray(np.repeat(w_bf.T, R, axis=0))     # [128, D]
    maskf = np.zeros((128, R), dtype=np.float32)
    for p in range(128):
        maskf[p, p % R] = 1.0
    rep = np.zeros((128, 128), dtype=ml_dtypes.bfloat16)
    for p in range(128):
        for m2 in range(NMETA):
            rep[p, m2 * R + (p % R)] = 1.0
    eye = np.eye(128, dtype=ml_dtypes.bfloat16)
    sidx = np.zeros((128, 56), dtype=np.int16)
    for p in range(128):
        for c in range(56):
            sidx[p, c] = c * R + (p % R)

    nc_raw = x_np.shape[1] // NCORES
    for core in range(NCORES):
        xs = x_np[:, core * nc_raw:(core + 1) * nc_raw, :]
        xp = np.zeros((NMETA, NC_PAD, D), dtype=ml_dtypes.bfloat16)
        xp[:, :nc_raw, :] = xs.astype(ml_dtypes.bfloat16)
        # xb[(m, r), (c, d)] = x[m, r*NB + c, d]
        xb = np.ascontiguousarray(
            xp.reshape(NMETA, R, NB * D).reshape(128, NB * D))
        in_maps.append({"x": xb, "wb": wb, "maskf": maskf, "rep": rep,
                        "eye": eye, "sidx": sidx})
    return in_maps


_CACHE = {}


def build(**kw):
    key = tuple(sorted((k, tuple(v) if isinstance(v, list) else v)
                       for k, v in kw.items()))
    if key in _CACHE:
        return _CACHE[key]
    nc = bacc.Bacc("TRN2", target_bir_lowering=False, debug=False,
                   num_devices=NCORES)
    x = nc.dram_tensor("x", [128, NB * D], BF16, kind="ExternalInput").ap()
    wb = nc.dram_tensor("wb", [128, D], BF16, kind="ExternalInput").ap()
    maskf = nc.dram_tensor("maskf", [128, R], F32, kind="ExternalInput").ap()
    rep = nc.dram_tensor("rep", [128, 128], BF16, kind="ExternalInput").ap()
    eye = nc.dram_tensor("eye", [128, 128], BF16, kind="ExternalInput").ap()
    sidx = nc.dram_tensor("sidx", [128, 56], I16, kind="ExternalInput").ap()
    # out is d-major: out[d, c*R + i] = y[node(r=i, c), d]
    out = nc.dram_tensor("out", [128, NB * R], BF16,
                         kind="ExternalOutput").ap()
    with tile.TileContext(nc) as tc:
        kernel_body(tc, out, x, wb, maskf, rep, eye, sidx, **kw)
    nc.compile()
    _CACHE[key] = nc
    return nc


def unpermute(o_core):
    # o_core [128=d, NB*R] with col j = c*R + i  ->  [NC_PAD, D], n = i*NB + c
    return np.ascontiguousarray(
        np.asarray(o_core).reshape(D, NB, R).transpose(2, 1, 0)
        .reshape(NC_PAD, D))


def run(input, W, trace=False, _build_kw=None, **trace_kwargs):
    x_np = np.asarray(input, dtype=np.float32)
    w_np = np.asarray(W, dtype=np.float32)
    nc = build(**(_build_kw or {}))
    in_maps = host_inputs(x_np, w_np)
    res = bass_utils.run_bass_kernel_spmd(
        nc, in_maps, core_ids=list(range(NCORES)), trace=trace, **trace_kwargs)
    nc_raw = x_np.shape[1] // NCORES
    full = np.concatenate(
        [unpermute(res.results[c]["out"])[:nc_raw] for c in range(NCORES)],
        axis=0).astype(np.float32)
    return full, res


def kernel(input, W):
    out, _ = run(input, W, trace=False)
    return out


# revision 34
# speedup vs baseline: 1.5383x; 1.0864x over previous
"""MetapathAttentionLayer Trainium2 kernel (v3).

Math (per node n):
    scores[n, m] = sum_d x[m, n, d] * W[d, m]
    att = softmax(relu(scores), axis=m)      (8 metapaths)
    out[n, :] = elu(sum_m att[n, m] * x[m, n, :])

Strategy: shard nodes across 8 cores (data parallel), 12544 padded nodes
per core. SBUF layout: partition p = (m, r), m = metapath (8), r =
node-residue (16); node n = r*784 + c for chunk-column c in [0, 784).

Per group of ~56 chunk-columns:
  - scores: DVE bf16 multiply (2x mode) then a d-reduction tree. The tree
    is split by chunk range: the first kd chunks run L1-L3 on DVE, the
    rest on GPSIMD(Pool) — balancing the two engines (both ~85us/core).
    L4-L7 run on DVE. relu+exp on ACT give e1 = exp(relu(s)) directly.
  - softmax denominator: one PE matmul with a residue-replication matrix
    sums e1 over m and replicates Z to all partitions; 1/Z via DVE fast
    reciprocal; att = e1 * invZ on DVE.
  - A-matrix (att * delta(r==i), layout [p, (i, c)]) via 16 ACT Copy ops
    with a per-partition 0/1 scale vector (ACT has slack).
  - pooling: one PE matmul per chunk (lhsT = X-chunk [128, 128d],
    rhs = strided A-slice [128, 16]) -> U[d, 16] in PSUM.
  - elu via PSUM accumulation: elu(u) = u + relu(-u) + exp(-relu(-u)) - 1.
    ACT computes b = relu(-U), c2 = exp(-b); two identity matmuls on PE
    accumulate b and c2 into the U bank; final ACT Copy applies bias -1
    and writes bf16.
  - store d-major [128=d, (c, i)] straight to DRAM (4KB-class descriptors,
    no transpose); host un-permutes to [n, d] for free.
"""

import numpy as np
import ml_dtypes

import concourse.bass as bass
import concourse.tile as tile
from concourse import bacc, mybir, library_config
import concourse.bass_utils as bass_utils

F32 = mybir.dt.float32
BF16 = mybir.dt.bfloat16
I16 = mybir.dt.int16
ALU = mybir.AluOpType
ACTF = mybir.ActivationFunctionType

NMETA = 8
N = 100000
D = 128
NCORES = 8
NC_RAW = N // NCORES          # 12500 nodes per core
R = 16                        # node-residue groups on partitions
NB = 784                      # chunk-columns per residue: 16*784 = 12544
NC_PAD = R * NB               # padded nodes per core


def kernel_body(tc, out_d, x_d, wb_d, maskf_d, rep_d, eye_d, sidx_d,
                mult_split=4, a_eng="pool", out_q="act",
                out_delay=3, tail_tight=0, bufs_x=4, bufs_p=2, bufs_tree=2,
                bufs_small=10, bufs_a=2, bufs_bc=2, bufs_y=4, sizes=None):
    nc = tc.nc
    if sizes is None:
        sizes = [56] * 14
    assert sum(sizes) == NB
    offs = [0]
    for s_ in sizes:
        offs.append(offs[-1] + s_)
    NGv = len(sizes)
    from contextlib import ExitStack
    with ExitStack() as ctx:
        const = ctx.enter_context(tc.tile_pool(name="const", bufs=1))
        xpool = ctx.enter_context(tc.tile_pool(name="x", bufs=bufs_x))
        ppool = ctx.enter_context(tc.tile_pool(name="p", bufs=bufs_p))
        tpool = ctx.enter_context(tc.tile_pool(name="tree", bufs=bufs_tree))
        spool = ctx.enter_context(tc.tile_pool(name="small", bufs=bufs_small))
        apool = ctx.enter_context(tc.tile_pool(name="amat", bufs=bufs_a))
        bcpool = ctx.enter_context(tc.tile_pool(name="bc", bufs=bufs_bc))
        ypool = ctx.enter_context(tc.tile_pool(name="y", bufs=bufs_y))
        psU = ctx.enter_context(tc.tile_pool(name="psU", bufs=2, space="PSUM"))
        psZ = ctx.enter_context(tc.tile_pool(name="psZ", bufs=2, space="PSUM"))
        psS = ctx.enter_context(tc.tile_pool(name="psS", bufs=2, space="PSUM"))

        cst = {}

        def load_consts():
            wb_t = const.tile([128, D], BF16, tag="wb")
            nc.sync.dma_start(wb_t[:], wb_d[:])
            maskf_t = const.tile([128, R], F32, tag="maskf")
            nc.sync.dma_start(maskf_t[:], maskf_d[:])
            rep_t = const.tile([128, 128], BF16, tag="rep")
            nc.sync.dma_start(rep_t[:], rep_d[:])
            eye_t = const.tile([128, 128], BF16, tag="eye")
            nc.sync.dma_start(eye_t[:], eye_d[:])
            sidx_t = const.tile([128, 56], I16, tag="sidx")
            nc.sync.dma_start(sidx_t[:], sidx_d[:])
            cst.update(wb=wb_t, maskf=maskf_t, rep=rep_t, eye=eye_t,
                       sidx=sidx_t)
            if a_eng != "dve":
                nc.gpsimd.load_library(library_config.local_scatter)

        dma_eng = {"act": nc.scalar, "sp": nc.sync, "pool": nc.gpsimd,
                   "dve": nc.vector}[out_q]
        st = [dict() for _ in range(NGv)]
        pending = []

        def load(g):
            ct = sizes[g]
            X = xpool.tile([128, ct * D], BF16, tag="X")
            nc.sync.dma_start(X[:], x_d[:, offs[g] * D:(offs[g] + ct) * D])
            st[g]["Xv"] = X[:].rearrange("p (c d) -> p c d", c=ct)

        def mult_scores(g):
            # scores: DVE bf16 multiply, then the d-reduction entirely on PE
            # (accumulate the 128 single-column slices into a [128, ct] f32
            # PSUM tile via identity matmuls). Split into `mult_split` chunk
            # ranges so PE can start while DVE is still multiplying.
            ct = sizes[g]
            Xv = st[g]["Xv"]
            P = ppool.tile([128, ct * D], BF16, tag="P")
            Pv = P[:].rearrange("p (c d) -> p c d", c=ct)
            S = psS.tile([128, ct], F32, tag="S")
            nh = max(1, min(mult_split, ct))
            step = (ct + nh - 1) // nh
            for c0 in range(0, ct, step):
                c1 = min(ct, c0 + step)
                nc.vector.tensor_tensor(
                    out=Pv[:, c0:c1, :], in0=Xv[:, c0:c1, :],
                    in1=cst["wb"][:].unsqueeze(1).broadcast_to(
                        [128, c1 - c0, D]),
                    op=ALU.mult)
                for k in range(128):
                    nc.tensor.matmul(out=S[:, c0:c1], lhsT=cst["eye"][:],
                                     rhs=Pv[:, c0:c1, k:k + 1],
                                     start=(k == 0), stop=(k == 127))
            st[g]["scores"] = S

        def softmax_head(g):
            ct = sizes[g]
            rs = spool.tile([128, ct], F32, tag="rs")
            nc.scalar.activation(rs[:], st[g].pop("scores")[:], ACTF.Relu)
            e1 = spool.tile([128, ct], BF16, tag="e1")
            nc.scalar.activation(e1[:], rs[:], ACTF.Exp)
            Z = psZ.tile([128, ct], F32, tag="Z")
            nc.tensor.matmul(out=Z[:], lhsT=cst["rep"][:], rhs=e1[:],
                             start=True, stop=True)
            st[g]["e1"] = e1
            st[g]["Z"] = Z

        def att_abuild(g):
            ct = sizes[g]
            inv = spool.tile([128, ct], F32, tag="inv")
            nc.vector.reciprocal_approx_fast(out=inv[:], in_=st[g].pop("Z")[:])
            att = spool.tile([128, ct], BF16, tag="att")
            nc.vector.tensor_tensor(out=att[:], in0=st[g].pop("e1")[:],
                                    in1=inv[:], op=ALU.mult)
            A = apool.tile([128, R * ct], BF16, tag="A")
            eng = a_eng if a_eng != "alt" else ("dve" if g % 2 == 0 else "pool")
            if eng == "dve":
                # A in (i, c) layout: A[p, i*ct + c] = att[p, c]*maskf[p, i]
                for i in range(R):
                    nc.vector.tensor_scalar(
                        A[:, i * ct:(i + 1) * ct], att[:],
                        cst["maskf"][:, i:i + 1], None, ALU.mult)
                st[g]["Av"] = A[:].rearrange("p (i c) -> p c i", i=R)
            else:
                # A in (c, i) layout via gpsimd scatter: A[p, c*R + r(p)]
                nc.gpsimd.local_scatter(A[:], att[:], cst["sidx"][:, 0:ct],
                                        channels=128, num_elems=ct * R,
                                        num_idxs=ct)
                st[g]["Av"] = A[:].rearrange("p (c i) -> p c i", i=R)

        def pool_mm(g):
            ct = sizes[g]
            Xv = st[g].pop("Xv")
            Av = st[g].pop("Av")
            U = psU.tile([128, ct * R], F32, tag="U")
            # Only the first matmul touching each 512-f32 PSUM bank may set
            # start=True (it lazily zeroes the whole 2KB zero region); the
            # ELU accumulates below add into live banks and close the group.
            for c in range(ct):
                nc.tensor.matmul(
                    out=U[:, c * R:(c + 1) * R],
                    lhsT=Xv[:, c, :],
                    rhs=Av[:, c, :],
                    start=(c * R) % 512 == 0, stop=False)
            st[g]["U"] = U

        def elu(g):
            # elu(U) = U + relu(-U) + exp(-relu(-U)) - 1: b and c2 are
            # accumulated into the U PSUM banks by identity matmuls (512-col
            # bank-aligned pieces; the last one closes the group), then the
            # final ACT Copy applies the -1 bias and writes bf16.
            ct = sizes[g]
            U = st[g].pop("U")
            b = bcpool.tile([128, ct * R], BF16, tag="b")
            nc.scalar.activation(b[:], U[:], ACTF.Relu, scale=-1.0)
            c2 = bcpool.tile([128, ct * R], BF16, tag="c2")
            nc.scalar.activation(c2[:], b[:], ACTF.Exp, scale=-1.0)
            for j in range(0, ct * R, 512):
                e_ = min(ct * R, j + 512)
                nc.tensor.matmul(out=U[:, j:e_], lhsT=cst["eye"][:],
                                 rhs=c2[:, j:e_], start=False, stop=False)
                nc.tensor.matmul(out=U[:, j:e_], lhsT=cst["eye"][:],
                                 rhs=b[:, j:e_], start=False, stop=True)
            y = ypool.tile([128, ct * R], BF16, tag="y")
            nc.scalar.activation(y[:], U[:], ACTF.Copy, bias=-1.0)
            pending.append((out_d[:, offs[g] * R:(offs[g] + ct) * R], y[:]))

        def ok(g):
            return 0 <= g < NGv

        load(0)
        load_consts()
        if ok(1):
            load(1)
        done = set()

        def back_half(g):
            if g in done or not ok(g):
                return
            done.add(g)
            att_abuild(g)
            pool_mm(g)
            elu(g)

        for it in range(NGv + 1):
            if ok(it + 2):
                load(it + 2)
            if pending and it >= out_delay:
                dst, src_ = pending.pop(0)
                dma_eng.dma_start(dst, src_)
            back_half(it - 1)
            if ok(it):
                mult_scores(it)
                softmax_head(it)
                if it >= NGv - tail_tight:
                    back_half(it)
        for dst, src_ in pending:
            dma_eng.dma_start(dst, src_)


def host_inputs(x_np, w_np):
    """Build per-core input maps from full fp32 inputs."""
    in_maps = []
    w_bf = w_np.astype(ml_dtypes.bfloat16)          # [D, NMETA]
    # wb[(m,r), d] = W[d, m]
    wb = np.ascontiguousarray(np.repeat(w_bf.T, R, axis=0))     # [128, D]
    maskf = np.zeros((128, R), dtype=np.float32)
    for p in range(128):
        maskf[p, p % R] = 1.0
    rep = np.zeros((128, 128), dtype=ml_dtypes.bfloat16)
    for p in range(128):
        for m2 in range(NMETA):
            rep[p, m2 * R + (p % R)] = 1.0
    eye = np.eye(128, dtype=ml_dtypes.bfloat16)
    sidx = np.zeros((128, 56), dtype=np.int16)
    for p in range(128):
        for c in range(56):
            sidx[p, c] = c * R + (p % R)

    nc_raw = x_np.shape[1] // NCORES
    for core in range(NCORES):
        xs = x_np[:, core * nc_raw:(core + 1) * nc_raw, :]
        xp = np.zeros((NMETA, NC_PAD, D), dtype=ml_dtypes.bfloat16)
        xp[:, :nc_raw, :] = xs.astype(ml_dtypes.bfloat16)
        # xb[(m, r), (c, d)] = x[m, r*NB + c, d]
        xb = np.ascontiguousarray(
            xp.reshape(NMETA, R, NB * D).reshape(128, NB * D))
        in_maps.append({"x": xb, "wb": wb, "maskf": maskf, "rep": rep,
                        "eye": eye, "sidx": sidx})
    return in_maps


_CACHE = {}


def build(**kw):
    key = tuple(sorted((k, tuple(v) if isinstance(v, list) else v)
                       for k, v in kw.items()))
    if key in _CACHE:
        return _CACHE[key]
    nc = bacc.Bacc("TRN2", target_bir_lowering=False, debug=False,
                   num_devices=NCORES)
    x = nc.dram_tensor("x", [128, NB * D], BF16, kind="ExternalInput").ap()
    wb = nc.dram_tensor("wb", [128, D], BF16, kind="ExternalInput").ap()
    maskf = nc.dram_tensor("maskf", [128, R], F32, kind="ExternalInput").ap()
    rep = nc.dram_tensor("rep", [128, 128], BF16, kind="ExternalInput").ap()
    eye = nc.dram_tensor("eye", [128, 128], BF16, kind="ExternalInput").ap()
    sidx = nc.dram_tensor("sidx", [128, 56], I16, kind="ExternalInput").ap()
    # out is d-major: out[d, c*R + i] = y[node(r=i, c), d]
    out = nc.dram_tensor("out", [128, NB * R], BF16,
                         kind="ExternalOutput").ap()
    with tile.TileContext(nc) as tc:
        kernel_body(tc, out, x, wb, maskf, rep, eye, sidx, **kw)
    nc.compile()
    _CACHE[key] = nc
    return nc


def unpermute(o_core):
    # o_core [128=d, NB*R] with col j = c*R + i  ->  [NC_PAD, D], n = i*NB + c
    return np.ascontiguousarray(
        np.asarray(o_core).reshape(D, NB, R).transpose(2, 1, 0)
        .reshape(NC_PAD, D))


def run(input, W, trace=False, _build_kw=None, **trace_kwargs):
    x_np = np.asarray(input, dtype=np.float32)
    w_np = np.asarray(W, dtype=np.float32)
    nc = build(**(_build_kw or {}))
    in_maps = host_inputs(x_np, w_np)
    res = bass_utils.run_bass_kernel_spmd(
        nc, in_maps, core_ids=list(range(NCORES)), trace=trace, **trace_kwargs)
    nc_raw = x_np.shape[1] // NCORES
    full = np.concatenate(
        [unpermute(res.results[c]["out"])[:nc_raw] for c in range(NCORES)],
        axis=0).astype(np.float32)
    return full, res


def kernel(input, W):
    out, _ = run(input, W, trace=False)
    return out


# revision 47
# speedup vs baseline: 1.5468x; 1.0055x over previous
"""MetapathAttentionLayer Trainium2 kernel (v3).

Math (per node n):
    scores[n, m] = sum_d x[m, n, d] * W[d, m]
    att = softmax(relu(scores), axis=m)      (8 metapaths)
    out[n, :] = elu(sum_m att[n, m] * x[m, n, :])

Strategy: shard nodes across 8 cores (data parallel), 12544 padded nodes
per core. SBUF layout: partition p = (m, r), m = metapath (8), r =
node-residue (16); node n = r*784 + c for chunk-column c in [0, 784).

Per group of ~56 chunk-columns:
  - scores: DVE bf16 multiply (2x mode) then a d-reduction tree. The tree
    is split by chunk range: the first kd chunks run L1-L3 on DVE, the
    rest on GPSIMD(Pool) — balancing the two engines (both ~85us/core).
    L4-L7 run on DVE. relu+exp on ACT give e1 = exp(relu(s)) directly.
  - softmax denominator: one PE matmul with a residue-replication matrix
    sums e1 over m and replicates Z to all partitions; 1/Z via DVE fast
    reciprocal; att = e1 * invZ on DVE.
  - A-matrix (att * delta(r==i), layout [p, (i, c)]) via 16 ACT Copy ops
    with a per-partition 0/1 scale vector (ACT has slack).
  - pooling: one PE matmul per chunk (lhsT = X-chunk [128, 128d],
    rhs = strided A-slice [128, 16]) -> U[d, 16] in PSUM.
  - elu via PSUM accumulation: elu(u) = u + relu(-u) + exp(-relu(-u)) - 1.
    ACT computes b = relu(-U), c2 = exp(-b); two identity matmuls on PE
    accumulate b and c2 into the U bank; final ACT Copy applies bias -1
    and writes bf16.
  - store d-major [128=d, (c, i)] straight to DRAM (4KB-class descriptors,
    no transpose); host un-permutes to [n, d] for free.
"""

import numpy as np
import ml_dtypes

import concourse.bass as bass
import concourse.tile as tile
from concourse import bacc, mybir, library_config
import concourse.bass_utils as bass_utils

F32 = mybir.dt.float32
BF16 = mybir.dt.bfloat16
I16 = mybir.dt.int16
ALU = mybir.AluOpType
ACTF = mybir.ActivationFunctionType

NMETA = 8
N = 100000
D = 128
NCORES = 8
NC_RAW = N // NCORES          # 12500 nodes per core
R = 16                        # node-residue groups on partitions
NB = 784                      # chunk-columns per residue: 16*784 = 12544
NC_PAD = R * NB               # padded nodes per core


def kernel_body(tc, out_d, x_d, wb_d, maskf_d, rep_d, eye_d, sidx_d,
                mult_split=4, dve_slices=0, a_eng="pool", y_comb="pe",
                out_q="act",
                out_delay=3, tail_tight=0, bufs_x=4, bufs_p=2, bufs_tree=2,
                bufs_small=10, bufs_a=2, bufs_bc=2, bufs_y=4, sizes=None):
    nc = tc.nc
    if sizes is None:
        sizes = [56] * 13 + [40, 16]
    assert sum(sizes) == NB
    offs = [0]
    for s_ in sizes:
        offs.append(offs[-1] + s_)
    NGv = len(sizes)
    from contextlib import ExitStack
    with ExitStack() as ctx:
        const = ctx.enter_context(tc.tile_pool(name="const", bufs=1))
        xpool = ctx.enter_context(tc.tile_pool(name="x", bufs=bufs_x))
        ppool = ctx.enter_context(tc.tile_pool(name="p", bufs=bufs_p))
        tpool = ctx.enter_context(tc.tile_pool(name="tree", bufs=bufs_tree))
        spool = ctx.enter_context(tc.tile_pool(name="small", bufs=bufs_small))
        apool = ctx.enter_context(tc.tile_pool(name="amat", bufs=bufs_a))
        bcpool = ctx.enter_context(tc.tile_pool(name="bc", bufs=bufs_bc))
        ypool = ctx.enter_context(tc.tile_pool(name="y", bufs=bufs_y))
        psU = ctx.enter_context(tc.tile_pool(name="psU", bufs=2, space="PSUM"))
        psZ = ctx.enter_context(tc.tile_pool(name="psZ", bufs=2, space="PSUM"))
        psS = ctx.enter_context(tc.tile_pool(name="psS", bufs=2, space="PSUM"))

        cst = {}

        def load_wb():
            wb_t = const.tile([128, D], BF16, tag="wb")
            nc.sync.dma_start(wb_t[:], wb_d[:])
            cst.update(wb=wb_t)

        def load_consts():
            eye_t = const.tile([128, 128], BF16, tag="eye")
            nc.sync.dma_start(eye_t[:], eye_d[:])
            maskf_t = const.tile([128, R], F32, tag="maskf")
            nc.sync.dma_start(maskf_t[:], maskf_d[:])
            rep_t = const.tile([128, 128], BF16, tag="rep")
            nc.sync.dma_start(rep_t[:], rep_d[:])
            sidx_t = const.tile([128, 56], I16, tag="sidx")
            nc.sync.dma_start(sidx_t[:], sidx_d[:])
            cst.update(maskf=maskf_t, rep=rep_t, eye=eye_t, sidx=sidx_t)
            if a_eng != "dve":
                nc.gpsimd.load_library(library_config.local_scatter)

        dma_eng = {"act": nc.scalar, "sp": nc.sync, "pool": nc.gpsimd,
                   "dve": nc.vector}[out_q]
        st = [dict() for _ in range(NGv)]
        pending = []

        def load(g):
            ct = sizes[g]
            X = xpool.tile([128, ct * D], BF16, tag="X")
            nc.sync.dma_start(X[:], x_d[:, offs[g] * D:(offs[g] + ct) * D])
            st[g]["Xv"] = X[:].rearrange("p (c d) -> p c d", c=ct)

        def mult_scores(g):
            # scores: DVE bf16 multiply, then d-reduction on PE (accumulate
            # single-column slices into a [128, ct] f32 PSUM tile via
            # identity matmuls). Split into `mult_split` chunk ranges so PE
            # starts while DVE is still multiplying. `dve_slices` d-pairs
            # are pre-added on DVE to offload PE.
            ct = sizes[g]
            Xv = st[g]["Xv"]
            P = ppool.tile([128, ct * D], BF16, tag="P")
            Pv = P[:].rearrange("p (c d) -> p c d", c=ct)
            ds = dve_slices
            if ds:
                PH = tpool.tile([128, ct * ds], BF16, tag="PH")
                PHv = PH[:].rearrange("p (c d) -> p c d", c=ct)
            S = psS.tile([128, ct], F32, tag="S")
            npass = 128 - ds
            nh = max(1, min(mult_split, ct))
            step = (ct + nh - 1) // nh
            for c0 in range(0, ct, step):
                c1 = min(ct, c0 + step)
                nc.vector.tensor_tensor(
                    out=Pv[:, c0:c1, :], in0=Xv[:, c0:c1, :],
                    in1=cst["wb"][:].unsqueeze(1).broadcast_to(
                        [128, c1 - c0, D]),
                    op=ALU.mult)
                if ds:
                    nc.vector.tensor_tensor(
                        out=PHv[:, c0:c1, :], in0=Pv[:, c0:c1, 0:ds],
                        in1=Pv[:, c0:c1, ds:2 * ds], op=ALU.add)
                for k in range(npass):
                    rhs = (PHv[:, c0:c1, k:k + 1] if k < ds
                           else Pv[:, c0:c1, ds + k:ds + k + 1])
                    nc.tensor.matmul(out=S[:, c0:c1], lhsT=cst["eye"][:],
                                     rhs=rhs,
                                     start=(k == 0), stop=(k == npass - 1))
            st[g]["scores"] = S

        def softmax_head(g):
            ct = sizes[g]
            rs = spool.tile([128, ct], F32, tag="rs")
            nc.scalar.activation(rs[:], st[g].pop("scores")[:], ACTF.Relu)
            e1 = spool.tile([128, ct], BF16, tag="e1")
            nc.scalar.activation(e1[:], rs[:], ACTF.Exp)
            Z = psZ.tile([128, ct], F32, tag="Z")
            nc.tensor.matmul(out=Z[:], lhsT=cst["rep"][:], rhs=e1[:],
                             start=True, stop=True)
            st[g]["e1"] = e1
            st[g]["Z"] = Z

        def att_abuild(g):
            ct = sizes[g]
            inv = spool.tile([128, ct], F32, tag="inv")
            nc.vector.reciprocal_approx_fast(out=inv[:], in_=st[g].pop("Z")[:])
            att = spool.tile([128, ct], BF16, tag="att")
            nc.vector.tensor_tensor(out=att[:], in0=st[g].pop("e1")[:],
                                    in1=inv[:], op=ALU.mult)
            A = apool.tile([128, R * ct], BF16, tag="A")
            eng = a_eng if a_eng != "alt" else ("dve" if g % 2 == 0 else "pool")
            if eng == "dve":
                # A in (i, c) layout: A[p, i*ct + c] = att[p, c]*maskf[p, i]
                for i in range(R):
                    nc.vector.tensor_scalar(
                        A[:, i * ct:(i + 1) * ct], att[:],
                        cst["maskf"][:, i:i + 1], None, ALU.mult)
                st[g]["Av"] = A[:].rearrange("p (i c) -> p c i", i=R)
            else:
                # A in (c, i) layout via gpsimd scatter: A[p, c*R + r(p)]
                nc.gpsimd.local_scatter(A[:], att[:], cst["sidx"][:, 0:ct],
                                        channels=128, num_elems=ct * R,
                                        num_idxs=ct)
                st[g]["Av"] = A[:].rearrange("p (c i) -> p c i", i=R)

        def pool_mm(g):
            ct = sizes[g]
            Xv = st[g].pop("Xv")
            Av = st[g].pop("Av")
            U = psU.tile([128, ct * R], F32, tag="U")
            if y_comb == "pe":
                # Accumulation group stays open for the ELU accumulates:
                # only the first matmul per 512-f32 PSUM bank sets start.
                for c in range(ct):
                    nc.tensor.matmul(
                        out=U[:, c * R:(c + 1) * R],
                        lhsT=Xv[:, c, :],
                        rhs=Av[:, c, :],
                        start=(c * R) % 512 == 0, stop=False)
            else:
                for c in range(ct):
                    nc.tensor.matmul(
                        out=U[:, c * R:(c + 1) * R],
                        lhsT=Xv[:, c, :],
                        rhs=Av[:, c, :],
                        start=True, stop=True)
            st[g]["U"] = U

        def elu(g):
            # elu(U) = relu(U) + exp(-relu(-U)) - 1.
            ct = sizes[g]
            U = st[g].pop("U")
            b = bcpool.tile([128, ct * R], BF16, tag="b")
            nc.scalar.activation(b[:], U[:], ACTF.Relu, scale=-1.0)
            c2 = bcpool.tile([128, ct * R], BF16, tag="c2")
            nc.scalar.activation(c2[:], b[:], ACTF.Exp, scale=-1.0)
            y = ypool.tile([128, ct * R], BF16, tag="y")
            if y_comb == "pe":
                # b and c2 are accumulated into the U PSUM banks by identity
                # matmuls (bank-aligned pieces close the group), then the
                # final ACT Copy applies -1: y = U + b + c2 - 1.
                for j in range(0, ct * R, 512):
                    e_ = min(ct * R, j + 512)
                    nc.tensor.matmul(out=U[:, j:e_], lhsT=cst["eye"][:],
                                     rhs=c2[:, j:e_], start=False, stop=False)
                    nc.tensor.matmul(out=U[:, j:e_], lhsT=cst["eye"][:],
                                     rhs=b[:, j:e_], start=False, stop=True)
                nc.scalar.activation(y[:], U[:], ACTF.Copy, bias=-1.0)
            else:
                a = bcpool.tile([128, ct * R], BF16, tag="a")
                nc.scalar.activation(a[:], U[:], ACTF.Relu)
                if y_comb == "dve1":
                    nc.vector.scalar_tensor_tensor(
                        out=y[:], in0=a[:], scalar=-1.0, in1=c2[:],
                        op0=ALU.add, op1=ALU.add)
                else:
                    s_ = bcpool.tile([128, ct * R], BF16, tag="s_")
                    nc.vector.tensor_tensor(out=s_[:], in0=a[:], in1=c2[:],
                                            op=ALU.add)
                    nc.vector.tensor_scalar(y[:], s_[:], -1.0, None, ALU.add)
            pending.append((out_d[:, offs[g] * R:(offs[g] + ct) * R], y[:]))

        def ok(g):
            return 0 <= g < NGv

        load(0)
        load_wb()
        load_consts()
        if ok(1):
            load(1)
        done = set()

        def back_half(g):
            if g in done or not ok(g):
                return
            done.add(g)
            att_abuild(g)
            pool_mm(g)
            elu(g)

        for it in range(NGv + 1):
            if ok(it + 2):
                load(it + 2)
            if pending and it >= out_delay:
                dst, src_ = pending.pop(0)
                dma_eng.dma_start(dst, src_)
            back_half(it - 1)
            if ok(it):
                mult_scores(it)
                softmax_head(it)
                if it >= NGv - tail_tight:
                    back_half(it)
        for dst, src_ in pending:
            dma_eng.dma_start(dst, src_)


def host_inputs(x_np, w_np):
    """Build per-core input maps from full fp32 inputs."""
    in_maps = []
    w_bf = w_np.astype(ml_dtypes.bfloat16)          # [D, NMETA]
    # wb[(m,r), d] = W[d, m]
    wb = np.ascontiguousarray(np.repeat(w_bf.T, R, axis=0))     # [128, D]
    maskf = np.zeros((128, R), dtype=np.float32)
    for p in range(128):
        maskf[p, p % R] = 1.0
    rep = np.zeros((128, 128), dtype=ml_dtypes.bfloat16)
    for p in range(128):
        for m2 in range(NMETA):
            rep[p, m2 * R + (p % R)] = 1.0
    eye = np.eye(128, dtype=ml_dtypes.bfloat16)
    sidx = np.zeros((128, 56), dtype=np.int16)
    for p in range(128):
        for c in range(56):
            sidx[p, c] = c * R + (p % R)

    nc_raw = x_np.shape[1] // NCORES
    for core in range(NCORES):
        xs = x_np[:, core * nc_raw:(core + 1) * nc_raw, :]
        xp = np.zeros((NMETA, NC_PAD, D), dtype=ml_dtypes.bfloat16)
        xp[:, :nc_raw, :] = xs.astype(ml_dtypes.bfloat16)
        # xb[(m, r), (c, d)] = x[m, r*NB + c, d]
        xb = np.ascontiguousarray(
            xp.reshape(NMETA, R, NB * D).reshape(128, NB * D))
        in_maps.append({"x": xb, "wb": wb, "maskf": maskf, "rep": rep,
                        "eye": eye, "sidx": sidx})
    return in_maps


_CACHE = {}


def build(**kw):
    key = tuple(sorted((k, tuple(v) if isinstance(v, list) else v)
                       for k, v in kw.items()))
    if key in _CACHE:
        return _CACHE[key]
    nc = bacc.Bacc("TRN2", target_bir_lowering=False, debug=False,
                   num_devices=NCORES)
    x = nc.dram_tensor("x", [128, NB * D], BF16, kind="ExternalInput").ap()
    wb = nc.dram_tensor("wb", [128, D], BF16, kind="ExternalInput").ap()
    maskf = nc.dram_tensor("maskf", [128, R], F32, kind="ExternalInput").ap()
    rep = nc.dram_tensor("rep", [128, 128], BF16, kind="ExternalInput").ap()
    eye = nc.dram_tensor("eye", [128, 128], BF16, kind="ExternalInput").ap()
    sidx = nc.dram_tensor("sidx", [128, 56], I16, kind="ExternalInput").ap()
    # out is d-major: out[d, c*R + i] = y[node(r=i, c), d]
    out = nc.dram_tensor("out", [128, NB * R], BF16,
                         kind="ExternalOutput").ap()
    with tile.TileContext(nc) as tc:
        kernel_body(tc, out, x, wb, maskf, rep, eye, sidx, **kw)
    nc.compile()
    _CACHE[key] = nc
    return nc


def unpermute(o_core):
    # o_core [128=d, NB*R] with col j = c*R + i  ->  [NC_PAD, D], n = i*NB + c
    return np.ascontiguousarray(
        np.asarray(o_core).reshape(D, NB, R).transpose(2, 1, 0)
        .reshape(NC_PAD, D))


def run(input, W, trace=False, _build_kw=None, **trace_kwargs):
    x_np = np.asarray(input, dtype=np.float32)
    w_np = np.asarray(W, dtype=np.float32)
    nc = build(**(_build_kw or {}))
    in_maps = host_inputs(x_np, w_np)
    res = bass_utils.run_bass_kernel_spmd(
        nc, in_maps, core_ids=list(range(NCORES)), trace=trace, **trace_kwargs)
    nc_raw = x_np.shape[1] // NCORES
    full = np.concatenate(
        [unpermute(res.results[c]["out"])[:nc_raw] for c in range(NCORES)],
        axis=0).astype(np.float32)
    return full, res


def kernel(input, W):
    out, _ = run(input, W, trace=False)
    return out
